# revision 1
# baseline (speedup 1.0000x reference)
"""GQA sliding-window attention (B=1, T=2048, C=2048, 32 Q / 8 KV heads,
head_dim=64, window=512, 16 global tokens) on 8 Trainium2 NeuronCores.

Sharding: tensor-parallel over heads — core c owns KV head c and Q heads
4c..4c+3.  Everything on-device runs transposed ([feature, token] layout):
  qT/kT = rotated projections with head dims pre-permuted to rotate-half
  order (evens then odds) so RoPE is two 32-partition half-swaps + 2 mul
  + 1 add; scores are computed as S^T = kT-block @ qT (keys on psum
  partitions) so softmax probs come out in the exact layout the PV matmul
  wants; V gets a ones-column appended so the PV matmul also produces the
  softmax denominator; y stays transposed which is the rhs layout of the
  output projection.  yT is AllGather'd across cores per 512-token chunk
  (overlapped with the next chunk's attention); each core computes a
  256-row slice of out^T one chunk behind; the host concatenates and
  transposes.

Matmuls run in float32r (TF32-class, full PE rate at moving-dim >= 256).
"""

import sys
sys.path.insert(0, "/opt/trn_rl_repo")

import numpy as np

import concourse.bass as bass
import concourse.mybir as mybir
from concourse import bacc
from concourse.tile import TileContext
from concourse.masks import make_identity

f32 = mybir.dt.float32
f32r = mybir.dt.float32r
AF = mybir.ActivationFunctionType

N_CORES = 8
T = 2048
C = 2048
HD = 64
NH_LOC = 4            # query heads per core
QD = NH_LOC * HD      # 256 per-core query dims
NB = T // 128         # 16 token blocks
NSB = T // 512        # 4 superblocks (one psum bank each)
N_GLOBAL = 16
SCALE = 0.125         # 1/sqrt(64)

_CACHE = {}


def _build():
    nc = bacc.Bacc(num_devices=N_CORES)

    xT = nc.declare_dram_parameter("xT", [C, T], f32r, isOutput=False)
    wqT = nc.declare_dram_parameter("wqT", [C, QD], f32r, isOutput=False)
    wkvT = nc.declare_dram_parameter("wkvT", [C, 128], f32r, isOutput=False)
    woT = nc.declare_dram_parameter("woT", [C, QD], f32r, isOutput=False)
    cs1 = nc.declare_dram_parameter("cs128", [128, T], f32, isOutput=False)
    sn1 = nc.declare_dram_parameter("sn128", [128, T], f32, isOutput=False)
    outT = nc.declare_dram_parameter("outT", [QD, T], f32, isOutput=True)

    with TileContext(nc) as tc:
        with tc.tile_pool(name="persist", bufs=1) as pp, \
             tc.tile_pool(name="psAll", bufs=1, space="PSUM") as psA, \
             tc.tile_pool(name="pdram", bufs=1, space="DRAM") as pdr:
            # ---- persistent state ----
            qTr01 = pp.tile([128, T], f32r)   # rotated q, heads 0,1
            qTr23 = pp.tile([128, T], f32r)   # rotated q, heads 2,3
            kTr2 = pp.tile([128, T], f32r)    # rotated k duplicated on both halves
            vT = pp.tile([64, T], f32)        # v (d, t) pre-transpose
            vgs = [pp.tile([128, HD + 1], f32r, name=f"vg{j}", tag=f"vg{j}")
                   for j in range(NB)]
            ytn = [pp.tile([64, T], f32r, name=f"ytn{h}", tag=f"ytn{h}")
                   for h in range(NH_LOC)]
            csb = pp.tile([128, T], f32)
            snb = pp.tile([128, T], f32)
            ident = pp.tile([128, 128], f32)
            m_diag = pp.tile([128, 128], f32)
            m_edge = pp.tile([128, 128], f32)
            mg16 = pp.tile([128, 1], f32)
            ones1 = pp.tile([128, 1], f32)

            make_identity(nc, ident[:])
            # m_diag keeps p <= f (causal; scoresT layout: partition=key,
            # free=query); m_edge keeps p > f (window lower edge)
            nc.gpsimd.memset(m_diag[:], 1.0)
            nc.gpsimd.affine_select(out=m_diag[:], in_=m_diag[:],
                                    compare_op=mybir.AluOpType.is_ge, fill=0.0,
                                    base=0, pattern=[[1, 128]],
                                    channel_multiplier=-1)
            nc.gpsimd.memset(m_edge[:], 1.0)
            nc.gpsimd.affine_select(out=m_edge[:], in_=m_edge[:],
                                    compare_op=mybir.AluOpType.is_ge, fill=0.0,
                                    base=-1, pattern=[[-1, 128]],
                                    channel_multiplier=1)
            # mg16: 0 for partitions < 16, else 1 (kills window copies of
            # global keys in k-block 0)
            nc.gpsimd.memset(mg16[:], 1.0)
            nc.gpsimd.affine_select(out=mg16[:], in_=mg16[:],
                                    compare_op=mybir.AluOpType.is_ge, fill=0.0,
                                    base=-N_GLOBAL, pattern=[[0, 1]],
                                    channel_multiplier=1)
            nc.vector.memset(ones1[:], 1.0)

            # psum tags shared across phases (8 banks total):
            #   q01/q23/kv/ptr (phase A) = yq/sw/sg/wp (attention + WO)
            def ps_tile(shape, tag, bufs=2):
                return psA.tile(shape, f32, tag=tag, bufs=bufs, name=tag,
                                padded_shape=[128, 512])

            # small always-live sbuf pool for attention stage tiles
            with tc.tile_pool(name="pbc", bufs=1) as pbc:
                # ============= phase A: QKV projections + RoPE =============
                with tc.tile_pool(name="pa", bufs=1) as pa:
                    wq_sb = [pa.tile([128, QD], f32r, name=f"wq{k}",
                                     tag=f"wq{k}") for k in range(16)]
                    wkv_sb = [pa.tile([128, 128], f32r, name=f"wkv{k}",
                                      tag=f"wkv{k}") for k in range(16)]

                    def rope(dst, psrc, rows, col0):
                        """dst[0:rows, col0:+512] = rope(psrc); head dims are
                        pre-permuted to rotate-half order."""
                        qs = pa.tile([rows, 512], f32, tag="qs", bufs=2)
                        for b in range(rows // 32):
                            s = b ^ 1
                            nc.vector.tensor_copy(qs[32 * b:32 * b + 32, :],
                                                  psrc[32 * s:32 * s + 32, :])
                        t1 = pa.tile([rows, 512], f32, tag="t1", bufs=2)
                        t2 = pa.tile([rows, 512], f32, tag="t2", bufs=2)
                        nc.vector.tensor_mul(t1[:], psrc,
                                             csb[0:rows, col0:col0 + 512])
                        nc.vector.tensor_mul(t2[:], qs[:],
                                             snb[0:rows, col0:col0 + 512])
                        nc.vector.tensor_add(dst[0:rows, col0:col0 + 512],
                                             t1[:], t2[:])

                    for tcc in range(NSB):
                        c0 = 512 * tcc
                        q01 = ps_tile([128, 512], "q01")
                        q23 = ps_tile([128, 512], "q23")
                        kv = ps_tile([128, 512], "kv")
                        for kt in range(16):
                            if tcc == 0:
                                nc.sync.dma_start(
                                    out=wq_sb[kt][:],
                                    in_=wqT[128 * kt:128 * (kt + 1), :])
                                nc.sync.dma_start(
                                    out=wkv_sb[kt][:],
                                    in_=wkvT[128 * kt:128 * (kt + 1), :])
                            xt = pa.tile([128, 512], f32r, tag="xt", bufs=8)
                            nc.sync.dma_start(
                                out=xt[:],
                                in_=xT[128 * kt:128 * (kt + 1), c0:c0 + 512])
                            if tcc == 0 and kt == 7:
                                nc.sync.dma_start(out=csb[:], in_=cs1[:])
                                nc.sync.dma_start(out=snb[:], in_=sn1[:])
                            st, sp = kt == 0, kt == 15
                            nc.tensor.matmul(q01[0:128, :],
                                             wq_sb[kt][:, 0:128],
                                             xt[:], start=st, stop=sp)
                            nc.tensor.matmul(q23[0:128, :],
                                             wq_sb[kt][:, 128:256],
                                             xt[:], start=st, stop=sp)
                            nc.tensor.matmul(kv[0:128, :], wkv_sb[kt][:],
                                             xt[:], start=st, stop=sp)
                        rope(qTr01, q01[0:128, :], 128, c0)
                        rope(qTr23, q23[0:128, :], 128, c0)
                        rope(kTr2, kv[0:64, :], 64, c0)
                        for b in range(2):
                            nc.vector.tensor_copy(
                                kTr2[64 + 32 * b:96 + 32 * b, c0:c0 + 512],
                                kTr2[32 * b:32 * (b + 1), c0:c0 + 512])
                        nc.vector.tensor_copy(vT[:, c0:c0 + 512],
                                              kv[64:128, :])
                        for j in range(4 * tcc, 4 * tcc + 4):
                            ptr = ps_tile([128, 64], "ptr")
                            nc.tensor.transpose(ptr[0:128, 0:64],
                                                vT[:, 128 * j:128 * (j + 1)],
                                                ident[0:64, 0:64])
                            nc.vector.tensor_copy(vgs[j][:, 0:HD],
                                                  ptr[0:128, 0:64])
                            nc.vector.tensor_copy(vgs[j][:, HD:HD + 1],
                                                  ones1[:])

                # ===== attention (Q-outer) + chunked AllGather + lagged WO ==
                with tc.tile_pool(name="pc", bufs=1) as pc:
                    wo_sb = [pc.tile([128, QD], f32r, name=f"wo{k}",
                                     tag=f"wo{k}") for k in range(16)]
                    for k in range(16):
                        nc.sync.dma_start(out=wo_sb[k][:],
                                          in_=woT[128 * k:128 * (k + 1), :])
                    agi = [pdr.tile([QD, 512], f32r, name=f"agi{Q}",
                                    tag=f"agi{Q}") for Q in range(NSB)]
                    ago = [pdr.tile([C, 512], f32r, name=f"ago{Q}",
                                    tag=f"ago{Q}", addr_space="Shared")
                           for Q in range(NSB)]

                    def attn(h, Q, den4):
                        qt = qTr01 if h < 2 else qTr23
                        qb = 64 * (h % 2)
                        c0 = 512 * Q
                        yq = ps_tile([HD + 1, 512], "q01")
                        sg = ps_tile([N_GLOBAL, 512], "q01")
                        nc.tensor.matmul(sg[0:N_GLOBAL, :],
                                         kTr2[qb:qb + 64, 0:N_GLOBAL],
                                         qt[qb:qb + 64, c0:c0 + 512],
                                         start=True, stop=True)
                        pg = pbc.tile([N_GLOBAL, 512], f32r, tag="pg", bufs=1)
                        nc.scalar.activation(pg[:], sg[0:N_GLOBAL, :],
                                             AF.Exp, scale=SCALE)
                        js = list(range(max(0, 4 * Q - 4), 4 * Q + 4))
                        DEPTH = 6
                        pts = {}

                        def scores(j):
                            qlo = max(4 * Q, j)
                            qhi = min(4 * Q + 3, j + 4)
                            s = 128 * (qhi - qlo + 1)
                            sw = ps_tile([128, s],
                                         ("kv", "q23", "ptr")[j % 3])
                            nc.tensor.matmul(
                                sw[0:128, 0:s],
                                kTr2[qb:qb + 64, 128 * j:128 * (j + 1)],
                                qt[qb:qb + 64, 128 * qlo:128 * qlo + s],
                                start=True, stop=True)
                            pt = pbc.tile([128, 512], f32r, tag="pt", bufs=7)
                            nc.scalar.activation(pt[:, 0:s], sw[0:128, 0:s],
                                                 AF.Exp, scale=SCALE)
                            if j == 0:
                                nc.vector.tensor_scalar_mul(pt[:, 0:s],
                                                            pt[:, 0:s],
                                                            mg16[:])
                            if j >= 4 * Q:       # causal diagonal block
                                nc.vector.tensor_mul(pt[:, 0:128],
                                                     pt[:, 0:128], m_diag[:])
                            else:                # window lower-edge block
                                nc.vector.tensor_mul(pt[:, s - 128:s],
                                                     pt[:, s - 128:s],
                                                     m_edge[:])
                            pts[j] = (pt, s, qlo)

                        def pv(j):
                            pt, s, qlo = pts.pop(j)
                            o = 128 * (qlo - 4 * Q)
                            nc.tensor.matmul(yq[0:HD + 1, o:o + s],
                                             vgs[j][:], pt[:, 0:s],
                                             start=False,
                                             stop=(j == 4 * Q + 3))

                        for idx in range(min(DEPTH, len(js))):
                            scores(js[idx])
                        nc.tensor.matmul(yq[0:HD + 1, :],
                                         vgs[0][0:N_GLOBAL, :], pg[:],
                                         start=True, stop=False)
                        for idx, j in enumerate(js):
                            if idx + DEPTH < len(js):
                                scores(js[idx + DEPTH])
                            pv(j)
                        nc.vector.tensor_copy(den4[h][:], yq[HD:HD + 1, :])
                        yu = pbc.tile([64, 512], f32, tag=f"yu{h}", bufs=1,
                                      name=f"yu{h}")
                        nc.vector.tensor_copy(yu[:], yq[0:HD, :])
                        yus.append(yu)

                    def wo_load(Q):
                        yts = []
                        for ci in range(16):
                            yt = pc.tile([128, 512], f32r, tag=f"yt{ci}",
                                         bufs=2, name=f"yt{ci}")
                            nc.sync.dma_start(
                                out=yt[:],
                                in_=ago[Q][128 * ci:128 * (ci + 1), :])
                            yts.append(yt)
                        return yts

                    def wo_chunk(Q, yts):
                        c0 = 512 * Q
                        for ob in range(2):
                            wp = ps_tile([128, 512],
                                         "q23" if ob == 0 else "kv")
                            for ci in range(16):
                                nc.tensor.matmul(
                                    wp[0:128, :],
                                    wo_sb[ci][:, 128 * ob:128 * (ob + 1)],
                                    yts[ci][:],
                                    start=(ci == 0), stop=(ci == 15))
                            ot = pc.tile([128, 512], f32, tag="ot", bufs=3)
                            nc.vector.tensor_copy(ot[:], wp[0:128, :])
                            nc.sync.dma_start(
                                out=outT[128 * ob:128 * (ob + 1),
                                         c0:c0 + 512],
                                in_=ot[:])

                    yt_pref = []
                    for Q in range(NSB):
                        c0 = 512 * Q
                        den4 = [pbc.tile([1, 512], f32, tag=f"den{h}",
                                         bufs=1, name=f"den{h}")
                                for h in range(NH_LOC)]
                        yus = []
                        for h in range(NH_LOC):
                            attn(h, Q, den4)
                        # 1/den as exp(-ln(den)) in place; grouping the Lns
                        # then the Exps costs one table-load pair per chunk
                        for h in range(NH_LOC):
                            nc.scalar.activation(den4[h][:], den4[h][:],
                                                 AF.Ln)
                        for h in range(NH_LOC):
                            nc.scalar.activation(den4[h][:], den4[h][:],
                                                 AF.Exp, scale=-1.0)
                        for h in range(NH_LOC):
                            rb = pbc.tile([64, 512], f32, tag="rb", bufs=1)
                            nc.gpsimd.partition_broadcast(rb[:],
                                                          den4[h][:])
                            nc.vector.tensor_mul(ytn[h][:, c0:c0 + 512],
                                                 yus[h][:], rb[:])
                            nc.sync.dma_start(
                                out=agi[Q][64 * h:64 * (h + 1), :],
                                in_=ytn[h][:, c0:c0 + 512])
                        nc.gpsimd.collective_compute(
                            "AllGather", mybir.AluOpType.bypass,
                            replica_groups=[list(range(N_CORES))],
                            ins=[agi[Q][:]], outs=[ago[Q][:]])
                        if Q >= 2:
                            yt_pref.append(wo_load(Q - 1))
                            wo_chunk(Q - 2, yt_pref.pop(0))
                        elif Q == 1:
                            yt_pref.append(wo_load(0))
                    yt_pref.append(wo_load(NSB - 1))
                    wo_chunk(NSB - 2, yt_pref.pop(0))
                    wo_chunk(NSB - 1, yt_pref.pop(0))

    nc.compile()
    return nc


_PERM = np.concatenate([np.arange(0, HD, 2), np.arange(1, HD, 2)])

# gathered-y row order is h-major: row 512h + 64c + d holds global channel
# 256c + 64h + d; permute wo's input dims to match
_CI_PERM = np.empty(C, np.int64)
for _h in range(NH_LOC):
    for _c in range(N_CORES):
        _CI_PERM[512 * _h + 64 * _c:512 * _h + 64 * _c + 64] = \
            np.arange(256 * _c + 64 * _h, 256 * _c + 64 * _h + 64)


def _prep_inputs(x, freqs_cos, freqs_sin, wq, wk, wv, wo):
    x = np.asarray(x, np.float32)
    wq = np.asarray(wq, np.float32)
    wk = np.asarray(wk, np.float32)
    wv = np.asarray(wv, np.float32)
    wo = np.asarray(wo, np.float32)
    fc = np.asarray(freqs_cos, np.float32).T   # [32, T]
    fs = np.asarray(freqs_sin, np.float32).T

    xT = np.ascontiguousarray(x[0].T)          # [C, T]
    cs128 = np.ascontiguousarray(np.concatenate([fc, fc, fc, fc], axis=0))
    sn128 = np.ascontiguousarray(np.concatenate([-fs, fs, -fs, fs], axis=0))

    in_maps = []
    for c in range(N_CORES):
        wq_c = wq[QD * c:QD * (c + 1), :].reshape(NH_LOC, HD, C)
        wq_c = wq_c[:, _PERM, :].reshape(QD, C)
        wk_c = wk[HD * c:HD * (c + 1), :][_PERM, :]
        wv_c = wv[HD * c:HD * (c + 1), :]
        in_maps.append({
            "xT": xT,
            "wqT": np.ascontiguousarray(wq_c.T),
            "wkvT": np.ascontiguousarray(
                np.concatenate([wk_c.T, wv_c.T], axis=1)),
            "woT": np.ascontiguousarray(wo[QD * c:QD * (c + 1), :].T),
            "cs128": cs128,
            "sn128": sn128,
        })
    return in_maps


def get_nc():
    if "nc" not in _CACHE:
        _CACHE["nc"] = _build()
    return _CACHE["nc"]


def kernel(x, freqs_cos, freqs_sin, wq, wk, wv, wo, **run_kwargs):
    from concourse.bass_utils import run_bass_kernel_spmd
    nc = get_nc()
    in_maps = _prep_inputs(x, freqs_cos, freqs_sin, wq, wk, wv, wo)
    res = run_bass_kernel_spmd(nc, in_maps, list(range(N_CORES)), **run_kwargs)
    outT = np.concatenate([res.results[c]["outT"] for c in range(N_CORES)],
                          axis=0)
    out = np.ascontiguousarray(outT.T).reshape(1, T, C).astype(np.float32)
    if run_kwargs:
        kernel.last_results = res
    return out



# revision 7
# speedup vs baseline: 1.2264x; 1.2264x over previous
"""GQA sliding-window attention (B=1, T=2048, C=2048, 32 Q / 8 KV heads,
head_dim=64, window=512, 16 global tokens) on 8 Trainium2 NeuronCores.

Sharding: tensor-parallel over heads — core c owns KV head c and Q heads
4c..4c+3.  Everything on-device runs transposed ([feature, token] layout):
  qT/kT = rotated projections with head dims pre-permuted to rotate-half
  order (evens then odds) so RoPE is two 32-partition half-swaps + 2 mul
  + 1 add; scores are computed as S^T = kT-block @ qT (keys on psum
  partitions) so softmax probs come out in the exact layout the PV matmul
  wants; V gets a ones-column appended so the PV matmul also produces the
  softmax denominator; y stays transposed which is the rhs layout of the
  output projection.  yT is AllGather'd across cores per 512-token chunk
  (overlapped with the next chunk's attention); each core computes a
  256-row slice of out^T one chunk behind; the host concatenates and
  transposes.

Matmuls/operands run in bf16 (psum accumulation stays f32) — same PE rate
as f32r but half the SBUF/DMA/collective bytes, which keeps the chip's
activity-based power throttle from duty-cycling the PE array.
"""

import sys
sys.path.insert(0, "/opt/trn_rl_repo")

import numpy as np

import concourse.bass as bass
import concourse.mybir as mybir
from concourse import bacc
from concourse.tile import TileContext
from concourse.masks import make_identity

f32 = mybir.dt.float32
bf16 = mybir.dt.bfloat16
AF = mybir.ActivationFunctionType

N_CORES = 8
T = 2048
C = 2048
HD = 64
NH_LOC = 4            # query heads per core
QD = NH_LOC * HD      # 256 per-core query dims
NB = T // 128         # 16 token blocks
NSB = T // 512        # 4 superblocks (one psum bank each)
N_GLOBAL = 16
SCALE = 0.125         # 1/sqrt(64)

_CACHE = {}


def _build():
    nc = bacc.Bacc(num_devices=N_CORES)

    xT = nc.declare_dram_parameter("xT", [C, T], bf16, isOutput=False)
    wqT = nc.declare_dram_parameter("wqT", [C, QD], bf16, isOutput=False)
    wkvT = nc.declare_dram_parameter("wkvT", [C, 128], bf16, isOutput=False)
    woT = nc.declare_dram_parameter("woT", [C, QD], bf16, isOutput=False)
    cs1 = nc.declare_dram_parameter("cs128", [128, T], f32, isOutput=False)
    sn1 = nc.declare_dram_parameter("sn128", [128, T], f32, isOutput=False)
    outT = nc.declare_dram_parameter("outT", [QD, T], f32, isOutput=True)

    with TileContext(nc) as tc:
        with tc.tile_pool(name="persist", bufs=1) as pp, \
             tc.tile_pool(name="psAll", bufs=1, space="PSUM") as psA, \
             tc.tile_pool(name="pdram", bufs=1, space="DRAM") as pdr:
            # ---- persistent state ----
            qTr01 = pp.tile([128, T], bf16)   # rotated q, heads 0,1
            qTr23 = pp.tile([128, T], bf16)   # rotated q, heads 2,3
            kTr2 = pp.tile([128, T], bf16)    # rotated k duplicated on both halves
            vT = pp.tile([64, T], bf16)        # v (d, t) pre-transpose
            vgs = [pp.tile([128, HD + 1], bf16, name=f"vg{j}", tag=f"vg{j}")
                   for j in range(NB)]
            ytn = [pp.tile([64, T], bf16, name=f"ytn{h}", tag=f"ytn{h}")
                   for h in range(NH_LOC)]
            csb = pp.tile([128, T], f32)
            snb = pp.tile([128, T], f32)
            ident = pp.tile([128, 128], bf16)
            m_diag = pp.tile([128, 128], bf16)
            m_edge = pp.tile([128, 128], bf16)
            mg16 = pp.tile([128, 1], f32)
            ones1 = pp.tile([128, 1], bf16)

            make_identity(nc, ident[:])
            # m_diag keeps p <= f (causal; scoresT layout: partition=key,
            # free=query); m_edge keeps p > f (window lower edge)
            nc.gpsimd.memset(m_diag[:], 1.0)
            nc.gpsimd.affine_select(out=m_diag[:], in_=m_diag[:],
                                    compare_op=mybir.AluOpType.is_ge, fill=0.0,
                                    base=0, pattern=[[1, 128]],
                                    channel_multiplier=-1)
            nc.gpsimd.memset(m_edge[:], 1.0)
            nc.gpsimd.affine_select(out=m_edge[:], in_=m_edge[:],
                                    compare_op=mybir.AluOpType.is_ge, fill=0.0,
                                    base=-1, pattern=[[-1, 128]],
                                    channel_multiplier=1)
            # mg16: 0 for partitions < 16, else 1 (kills window copies of
            # global keys in k-block 0)
            nc.gpsimd.memset(mg16[:], 1.0)
            nc.gpsimd.affine_select(out=mg16[:], in_=mg16[:],
                                    compare_op=mybir.AluOpType.is_ge, fill=0.0,
                                    base=-N_GLOBAL, pattern=[[0, 1]],
                                    channel_multiplier=1)
            nc.vector.memset(ones1[:], 1.0)

            # psum tags shared across phases (8 banks total):
            #   q01/q23/kv/ptr (phase A) = yq/sw/sg/wp (attention + WO)
            def ps_tile(shape, tag, bufs=2, dt=f32):
                return psA.tile(shape, dt, tag=tag, bufs=bufs, name=tag,
                                padded_shape=[128, 512])

            # small always-live sbuf pool for attention stage tiles
            with tc.tile_pool(name="pbc", bufs=1) as pbc:
                # ============= phase A: QKV projections + RoPE =============
                with tc.tile_pool(name="pa", bufs=1) as pa:
                    wq_sb = [pa.tile([128, QD], bf16, name=f"wq{k}",
                                     tag=f"wq{k}") for k in range(16)]
                    wkv_sb = [pa.tile([128, 128], bf16, name=f"wkv{k}",
                                      tag=f"wkv{k}") for k in range(16)]

                    def rope(dst, psrc, rows, col0):
                        """dst[0:rows, col0:+512] = rope(psrc); head dims are
                        pre-permuted to rotate-half order."""
                        qs = pa.tile([rows, 512], f32, tag="qs", bufs=2)
                        for b in range(rows // 32):
                            s = b ^ 1
                            nc.vector.tensor_copy(qs[32 * b:32 * b + 32, :],
                                                  psrc[32 * s:32 * s + 32, :])
                        t1 = pa.tile([rows, 512], f32, tag="t1", bufs=2)
                        t2 = pa.tile([rows, 512], f32, tag="t2", bufs=2)
                        nc.vector.tensor_mul(t1[:], psrc,
                                             csb[0:rows, col0:col0 + 512])
                        nc.vector.tensor_mul(t2[:], qs[:],
                                             snb[0:rows, col0:col0 + 512])
                        nc.vector.tensor_add(dst[0:rows, col0:col0 + 512],
                                             t1[:], t2[:])

                    for tcc in range(NSB):
                        c0 = 512 * tcc
                        q01 = ps_tile([128, 512], "q01")
                        q23 = ps_tile([128, 512], "q23")
                        kv = ps_tile([128, 512], "kv")
                        for kt in range(16):
                            if tcc == 0:
                                nc.sync.dma_start(
                                    out=wq_sb[kt][:],
                                    in_=wqT[128 * kt:128 * (kt + 1), :])
                                nc.sync.dma_start(
                                    out=wkv_sb[kt][:],
                                    in_=wkvT[128 * kt:128 * (kt + 1), :])
                            xt = pa.tile([128, 512], bf16, tag="xt", bufs=8)
                            nc.sync.dma_start(
                                out=xt[:],
                                in_=xT[128 * kt:128 * (kt + 1), c0:c0 + 512])
                            if tcc == 0 and kt == 7:
                                nc.sync.dma_start(out=csb[:], in_=cs1[:])
                                nc.sync.dma_start(out=snb[:], in_=sn1[:])
                            st, sp = kt == 0, kt == 15
                            nc.tensor.matmul(q01[0:128, :],
                                             wq_sb[kt][:, 0:128],
                                             xt[:], start=st, stop=sp)
                            nc.tensor.matmul(q23[0:128, :],
                                             wq_sb[kt][:, 128:256],
                                             xt[:], start=st, stop=sp)
                            nc.tensor.matmul(kv[0:128, :], wkv_sb[kt][:],
                                             xt[:], start=st, stop=sp)
                        rope(qTr01, q01[0:128, :], 128, c0)
                        rope(qTr23, q23[0:128, :], 128, c0)
                        rope(kTr2, kv[0:64, :], 64, c0)
                        for b in range(2):
                            nc.vector.tensor_copy(
                                kTr2[64 + 32 * b:96 + 32 * b, c0:c0 + 512],
                                kTr2[32 * b:32 * (b + 1), c0:c0 + 512])
                        nc.vector.tensor_copy(vT[:, c0:c0 + 512],
                                              kv[64:128, :])
                        for j in range(4 * tcc, 4 * tcc + 4):
                            ptr = ps_tile([128, 64], "ptr", dt=bf16)
                            nc.tensor.transpose(ptr[0:128, 0:64],
                                                vT[:, 128 * j:128 * (j + 1)],
                                                ident[0:64, 0:64])
                            nc.vector.tensor_copy(vgs[j][:, 0:HD],
                                                  ptr[0:128, 0:64])
                            nc.vector.tensor_copy(vgs[j][:, HD:HD + 1],
                                                  ones1[:])

                # ===== attention (Q-outer) + chunked AllGather + lagged WO ==
                with tc.tile_pool(name="pc", bufs=1) as pc:
                    wo_sb = [pc.tile([128, QD], bf16, name=f"wo{k}",
                                     tag=f"wo{k}") for k in range(16)]
                    for k in range(16):
                        nc.sync.dma_start(out=wo_sb[k][:],
                                          in_=woT[128 * k:128 * (k + 1), :])
                    agi = [pdr.tile([QD, 512], bf16, name=f"agi{Q}",
                                    tag=f"agi{Q}") for Q in range(NSB)]
                    ago = [pdr.tile([C, 512], bf16, name=f"ago{Q}",
                                    tag=f"ago{Q}", addr_space="Shared")
                           for Q in range(NSB)]

                    def attn(h, Q, den4):
                        qt = qTr01 if h < 2 else qTr23
                        qb = 64 * (h % 2)
                        c0 = 512 * Q
                        yq = ps_tile([HD + 1, 512], "q01")
                        sg = ps_tile([N_GLOBAL, 512], "q01")
                        nc.tensor.matmul(sg[0:N_GLOBAL, :],
                                         kTr2[qb:qb + 64, 0:N_GLOBAL],
                                         qt[qb:qb + 64, c0:c0 + 512],
                                         start=True, stop=True)
                        pg = pbc.tile([N_GLOBAL, 512], bf16, tag="pg", bufs=1)
                        nc.scalar.activation(pg[:], sg[0:N_GLOBAL, :],
                                             AF.Exp, scale=SCALE)
                        js = list(range(max(0, 4 * Q - 4), 4 * Q + 4))
                        DEPTH = 6
                        pts = {}

                        def scores(j):
                            qlo = max(4 * Q, j)
                            qhi = min(4 * Q + 3, j + 4)
                            s = 128 * (qhi - qlo + 1)
                            sw = ps_tile([128, s],
                                         ("kv", "q23", "ptr")[j % 3])
                            nc.tensor.matmul(
                                sw[0:128, 0:s],
                                kTr2[qb:qb + 64, 128 * j:128 * (j + 1)],
                                qt[qb:qb + 64, 128 * qlo:128 * qlo + s],
                                start=True, stop=True)
                            pt = pbc.tile([128, 512], bf16, tag="pt", bufs=7)
                            nc.scalar.activation(pt[:, 0:s], sw[0:128, 0:s],
                                                 AF.Exp, scale=SCALE)
                            if j == 0:
                                nc.vector.tensor_scalar_mul(pt[:, 0:s],
                                                            pt[:, 0:s],
                                                            mg16[:])
                            if j >= 4 * Q:       # causal diagonal block
                                nc.vector.tensor_mul(pt[:, 0:128],
                                                     pt[:, 0:128], m_diag[:])
                            else:                # window lower-edge block
                                nc.vector.tensor_mul(pt[:, s - 128:s],
                                                     pt[:, s - 128:s],
                                                     m_edge[:])
                            pts[j] = (pt, s, qlo)

                        def pv(j):
                            pt, s, qlo = pts.pop(j)
                            o = 128 * (qlo - 4 * Q)
                            nc.tensor.matmul(yq[0:HD + 1, o:o + s],
                                             vgs[j][:], pt[:, 0:s],
                                             start=False,
                                             stop=(j == 4 * Q + 3))

                        for idx in range(min(DEPTH, len(js))):
                            scores(js[idx])
                        nc.tensor.matmul(yq[0:HD + 1, :],
                                         vgs[0][0:N_GLOBAL, :], pg[:],
                                         start=True, stop=False)
                        for idx, j in enumerate(js):
                            if idx + DEPTH < len(js):
                                scores(js[idx + DEPTH])
                            pv(j)
                        nc.vector.tensor_copy(den4[h][:], yq[HD:HD + 1, :])
                        yu = pbc.tile([64, 512], f32, tag=f"yu{h}", bufs=1,
                                      name=f"yu{h}")
                        nc.vector.tensor_copy(yu[:], yq[0:HD, :])
                        yus.append(yu)

                    def wo_load(Q):
                        yts = []
                        for ci in range(16):
                            yt = pc.tile([128, 512], bf16, tag=f"yt{ci}",
                                         bufs=2, name=f"yt{ci}")
                            nc.sync.dma_start(
                                out=yt[:],
                                in_=ago[Q][128 * ci:128 * (ci + 1), :])
                            yts.append(yt)
                        return yts

                    def wo_chunk(Q, yts):
                        c0 = 512 * Q
                        for ob in range(2):
                            wp = ps_tile([128, 512],
                                         "q23" if ob == 0 else "kv")
                            for ci in range(16):
                                nc.tensor.matmul(
                                    wp[0:128, :],
                                    wo_sb[ci][:, 128 * ob:128 * (ob + 1)],
                                    yts[ci][:],
                                    start=(ci == 0), stop=(ci == 15))
                            ot = pc.tile([128, 512], f32, tag="ot", bufs=3)
                            nc.vector.tensor_copy(ot[:], wp[0:128, :])
                            nc.sync.dma_start(
                                out=outT[128 * ob:128 * (ob + 1),
                                         c0:c0 + 512],
                                in_=ot[:])

                    yt_pref = []
                    for Q in range(NSB):
                        c0 = 512 * Q
                        den4 = [pbc.tile([1, 512], f32, tag=f"den{h}",
                                         bufs=1, name=f"den{h}")
                                for h in range(NH_LOC)]
                        yus = []
                        for h in range(NH_LOC):
                            attn(h, Q, den4)
                        # 1/den as exp(-ln(den)) in place; grouping the Lns
                        # then the Exps costs one table-load pair per chunk
                        for h in range(NH_LOC):
                            nc.scalar.activation(den4[h][:], den4[h][:],
                                                 AF.Ln)
                        for h in range(NH_LOC):
                            nc.scalar.activation(den4[h][:], den4[h][:],
                                                 AF.Exp, scale=-1.0)
                        for h in range(NH_LOC):
                            rb = pbc.tile([64, 512], f32, tag="rb", bufs=1)
                            nc.gpsimd.partition_broadcast(rb[:],
                                                          den4[h][:])
                            nc.vector.tensor_mul(ytn[h][:, c0:c0 + 512],
                                                 yus[h][:], rb[:])
                            nc.sync.dma_start(
                                out=agi[Q][64 * h:64 * (h + 1), :],
                                in_=ytn[h][:, c0:c0 + 512])
                        nc.gpsimd.collective_compute(
                            "AllGather", mybir.AluOpType.bypass,
                            replica_groups=[list(range(N_CORES))],
                            ins=[agi[Q][:]], outs=[ago[Q][:]])
                        if Q >= 2:
                            yt_pref.append(wo_load(Q - 1))
                            wo_chunk(Q - 2, yt_pref.pop(0))
                        elif Q == 1:
                            yt_pref.append(wo_load(0))
                    yt_pref.append(wo_load(NSB - 1))
                    wo_chunk(NSB - 2, yt_pref.pop(0))
                    wo_chunk(NSB - 1, yt_pref.pop(0))

    nc.compile()
    return nc


_PERM = np.concatenate([np.arange(0, HD, 2), np.arange(1, HD, 2)])

# gathered-y row order is h-major: row 512h + 64c + d holds global channel
# 256c + 64h + d; permute wo's input dims to match
_CI_PERM = np.empty(C, np.int64)
for _h in range(NH_LOC):
    for _c in range(N_CORES):
        _CI_PERM[512 * _h + 64 * _c:512 * _h + 64 * _c + 64] = \
            np.arange(256 * _c + 64 * _h, 256 * _c + 64 * _h + 64)


def _prep_inputs(x, freqs_cos, freqs_sin, wq, wk, wv, wo):
    import ml_dtypes
    nbf = ml_dtypes.bfloat16
    x = np.asarray(x, np.float32)
    wq = np.asarray(wq, np.float32)
    wk = np.asarray(wk, np.float32)
    wv = np.asarray(wv, np.float32)
    wo = np.asarray(wo, np.float32)
    fc = np.asarray(freqs_cos, np.float32).T   # [32, T]
    fs = np.asarray(freqs_sin, np.float32).T

    xT = np.ascontiguousarray(x[0].T).astype(nbf)   # [C, T]
    cs128 = np.ascontiguousarray(np.concatenate([fc, fc, fc, fc], axis=0))
    sn128 = np.ascontiguousarray(np.concatenate([-fs, fs, -fs, fs], axis=0))

    in_maps = []
    for c in range(N_CORES):
        wq_c = wq[QD * c:QD * (c + 1), :].reshape(NH_LOC, HD, C)
        wq_c = wq_c[:, _PERM, :].reshape(QD, C)
        wk_c = wk[HD * c:HD * (c + 1), :][_PERM, :]
        wv_c = wv[HD * c:HD * (c + 1), :]
        in_maps.append({
            "xT": xT,
            "wqT": np.ascontiguousarray(wq_c.T).astype(nbf),
            "wkvT": np.ascontiguousarray(
                np.concatenate([wk_c.T, wv_c.T], axis=1)).astype(nbf),
            "woT": np.ascontiguousarray(wo[QD * c:QD * (c + 1), :].T).astype(nbf),
            "cs128": cs128,
            "sn128": sn128,
        })
    return in_maps


def get_nc():
    if "nc" not in _CACHE:
        _CACHE["nc"] = _build()
    return _CACHE["nc"]


def kernel(x, freqs_cos, freqs_sin, wq, wk, wv, wo, **run_kwargs):
    from concourse.bass_utils import run_bass_kernel_spmd
    nc = get_nc()
    in_maps = _prep_inputs(x, freqs_cos, freqs_sin, wq, wk, wv, wo)
    res = run_bass_kernel_spmd(nc, in_maps, list(range(N_CORES)), **run_kwargs)
    outT = np.concatenate([res.results[c]["outT"] for c in range(N_CORES)],
                          axis=0)
    out = np.ascontiguousarray(outT.T).reshape(1, T, C).astype(np.float32)
    if run_kwargs:
        kernel.last_results = res
    return out

